# revision 1
# baseline (speedup 1.0000x reference)
"""Trainium2 Bass kernel for the MemoryEfficientMambaBlock problem.

Data-parallel over 8 NeuronCores: x sharded over tokens, small weights
replicated. Per core, per 256-token tile:
  LayerNorm (bn_stats, token-major) -> PE transpose to feature-major
  (gamma/beta fused into the PSUM copyback) -> f32r matmul x@W_projT with
  SiLU+b_proj fused into the ACT copyback -> f32r matmul @W_stateT with
  SiLU+(b_state+initial_state) fused -> K=9 f32r matmul (ones row carries
  b_out) producing token-major output with the residual add fused into the
  DVE copyback.
"""

import sys

if "/opt/trn_rl_repo" not in sys.path:
    sys.path.insert(0, "/opt/trn_rl_repo")

import numpy as np

import concourse.bass as bass
import concourse.mybir as mybir
import concourse.tile as tile
from concourse.bass_utils import run_bass_kernel_spmd
from concourse.masks import make_identity

P = 128
D_MODEL = 1024
D_INNER = 2048
D_STATE = 8
EPS = 1e-5
N_CORES = 8
TOK_TOTAL = 2 * 128 * 196  # 50176
TOK = TOK_TOTAL // N_CORES  # 6272
TILE_T = 256

KD = D_MODEL // P  # 8 contraction chunks for matmul 1
ME = D_INNER // P  # 16 output tiles for matmul 1 / contraction chunks for 2

F32 = mybir.dt.float32
F32R = mybir.dt.float32r


def _split_multi_waits(nc):
    """This container's walrus accepts at most ONE semaphore wait per
    instruction. Hoist all but the last wait of each instruction onto
    fresh single-wait NoOps inserted immediately before it on the same
    engine (the sequencer processes instructions in order, so semantics
    are unchanged)."""
    n_split = 0
    for f in nc.m.functions:
        for blk in f.blocks:
            out = []
            changed = False
            for inst in blk.instructions:
                si = inst.sync_info
                waits = list(si.on_wait) if si is not None else []
                if len(waits) > 1:
                    changed = True
                    for j, w in enumerate(waits[:-1]):
                        nop = mybir.InstNoOp(
                            name=f"{inst.name}-wsplit{j}", ins=[], outs=[]
                        )
                        nop.engine = inst.engine
                        nop.sync_info = mybir.SyncInfo(on_wait=[w], on_update=[])
                        out.append(nop)
                        n_split += 1
                    inst.sync_info = mybir.SyncInfo(
                        on_wait=[waits[-1]], on_update=list(si.on_update)
                    )
                out.append(inst)
            if changed:
                blk.instructions = out
    return n_split


def build_kernel():
    nc = bass.Bass()
    x = nc.dram_tensor("x", [TOK, D_MODEL], F32, kind="ExternalInput")
    wpt = nc.dram_tensor("wpt", [D_MODEL, D_INNER], F32R, kind="ExternalInput")
    wst = nc.dram_tensor("wst", [D_INNER, D_STATE], F32R, kind="ExternalInput")
    wo9 = nc.dram_tensor("wo9", [D_STATE + 1, D_MODEL], F32R, kind="ExternalInput")
    gpk = nc.dram_tensor("gpk", [P, KD], F32, kind="ExternalInput")
    bpk = nc.dram_tensor("bpk", [P, KD], F32, kind="ExternalInput")
    bpm = nc.dram_tensor("bpm", [P, ME], F32, kind="ExternalInput")
    b2 = nc.dram_tensor("b2", [D_STATE, 1], F32, kind="ExternalInput")
    ones = nc.dram_tensor("ones", [1, TILE_T], F32R, kind="ExternalInput")
    ident_d = nc.dram_tensor("ident", [P, P], F32R, kind="ExternalInput")
    y = nc.dram_tensor("y", [TOK, D_MODEL], F32, kind="ExternalOutput")

    # all tiles full-size; the last tile overlaps the previous one so the
    # f32r matmuls always stream N>=256 (N<256 runs at 1/4 rate)
    tiles = [(o, TILE_T) for o in range(0, TOK - TILE_T + 1, TILE_T)]
    if tiles[-1][0] + TILE_T < TOK:
        tiles.append((TOK - TILE_T, TILE_T))

    with tile.TileContext(nc) as tc:
        with (
            tc.tile_pool(name="singles", bufs=1) as singles,
            tc.tile_pool(name="xpool", bufs=3) as xpool,
            tc.tile_pool(name="xnpool", bufs=2) as xnpool,
            tc.tile_pool(name="xtpool", bufs=2) as xtpool,
            tc.tile_pool(name="projp", bufs=2) as projp,
            tc.tile_pool(name="outp", bufs=2) as outp,
            tc.tile_pool(name="statp", bufs=6) as statp,
            tc.tile_pool(name="ps_tr", bufs=2, space="PSUM") as ps_tr,
            tc.tile_pool(name="ps1", bufs=3, space="PSUM") as ps1,
            tc.tile_pool(name="ps2", bufs=1, space="PSUM") as ps2,
            tc.tile_pool(name="ps3", bufs=1, space="PSUM") as ps3,
        ):
            wpt_sb = singles.tile([P, KD, D_INNER], F32R)
            wpt_r = wpt[:, :].rearrange("(k p) e -> p k e", p=P)
            for k in range(KD):
                # split across queues/engines so the 8MB load parallelizes
                eng = nc.sync if k % 2 == 0 else nc.scalar
                eng.dma_start(wpt_sb[:, k], wpt_r[:, k])
            wst_sb = singles.tile([P, ME, D_STATE], F32R)
            nc.sync.dma_start(wst_sb, wst[:, :].rearrange("(k p) s -> p k s", p=P))
            wo9_sb = singles.tile([D_STATE + 1, D_MODEL], F32R)
            nc.sync.dma_start(wo9_sb, wo9[:, :])
            gpk_sb = singles.tile([P, KD], F32)
            nc.sync.dma_start(gpk_sb, gpk[:, :])
            bpk_sb = singles.tile([P, KD], F32)
            nc.sync.dma_start(bpk_sb, bpk[:, :])
            bpm_sb = singles.tile([P, ME], F32)
            nc.sync.dma_start(bpm_sb, bpm[:, :])
            b2_sb = singles.tile([D_STATE, 1], F32)
            nc.sync.dma_start(b2_sb, b2[:, :])
            ident = singles.tile([P, P], F32R)
            nc.sync.dma_start(ident, ident_d[:, :])
            eps_sb = singles.tile([P, 1], F32)
            nc.vector.memset(eps_sb, EPS)

            def a_dma(off, T):
                G = T // P
                x_sb = xpool.tile([P, G, D_MODEL], F32, tag="x")
                nc.sync.dma_start(
                    x_sb, x[off : off + T, :].rearrange("(g p) d -> p g d", p=P)
                )
                return x_sb

            def a_ln(x_sb, T):
                """layernorm one loaded tile -> xn (token-major)"""
                G = T // P
                xn_sb = xnpool.tile([P, G, D_MODEL], F32R, tag="xn")
                for g in range(G):
                    stats = statp.tile([P, 2, 6], F32, tag="bnst")
                    nc.vector.bn_stats(stats[:, 0, :], x_sb[:, g, 0:512])
                    nc.vector.bn_stats(stats[:, 1, :], x_sb[:, g, 512:1024])
                    mv = statp.tile([P, 2], F32, tag="mv")
                    nc.vector.bn_aggr(mv, stats)
                    rstd = statp.tile([P, 1], F32, tag="rstd")
                    nc.scalar.activation(
                        rstd,
                        mv[:, 1:2],
                        mybir.ActivationFunctionType.Sqrt,
                        bias=eps_sb,
                    )
                    nc.vector.reciprocal(rstd, rstd)
                    nc.vector.tensor_scalar(
                        out=xn_sb[:, g, :],
                        in0=x_sb[:, g, :],
                        scalar1=mv[:, 0:1],
                        scalar2=rstd,
                        op0=mybir.AluOpType.subtract,
                        op1=mybir.AluOpType.mult,
                    )
                return xn_sb

            def a_tr(xn_sb, T):
                """PE-transpose to feature-major (f32r single-pass mode);
                gamma/beta fused into the PSUM copyback"""
                G = T // P
                xnT = xtpool.tile([P, KD, G, P], F32R, tag="xnT")
                for k in range(KD):
                    ptr = ps_tr.tile([P, G, P], F32R, tag="ptr")
                    for g in range(G):
                        nc.tensor.transpose(
                            ptr[:, g, :],
                            xn_sb[:, g, k * P : (k + 1) * P],
                            ident,
                        )
                    nc.vector.tensor_scalar(
                        out=xnT[:, k],
                        in0=ptr[:],
                        scalar1=gpk_sb[:, k : k + 1],
                        scalar2=bpk_sb[:, k : k + 1],
                        op0=mybir.AluOpType.mult,
                        op1=mybir.AluOpType.add,
                    )
                return xnT

            # software pipeline: x-DMA two tiles ahead, LayerNorm one tile
            # ahead (on DVE during this tile's matmul-1), transposes one tile
            # ahead in the M2->M3 ACT-latency pocket
            x_tiles = [a_dma(*tiles[0]), a_dma(*tiles[1])]
            xn_cur = a_ln(x_tiles[0], tiles[0][1])
            xnT_cur = a_tr(xn_cur, tiles[0][1])
            xn_next = a_ln(x_tiles[1], tiles[1][1])
            for i, (off, T) in enumerate(tiles):
                x_sb = x_tiles[i]
                xnT = xnT_cur
                G = T // P
                if i + 2 < len(tiles):
                    x_tiles.append(a_dma(*tiles[i + 2]))
                # cs9 allocated + ones row DMA'd early (row 8 is only
                # reachable by DMA; issuing here hides its latency)
                cs9 = statp.tile([D_STATE + 1, TILE_T], F32R, tag="cs9")
                nc.sync.dma_start(cs9[D_STATE : D_STATE + 1, :], ones[:, :])
                # matmul 1: [D_INNER, T] feature-major; SiLU+b_proj fused
                projT = projp.tile([P, ME, TILE_T], F32R, tag="projT")
                for m in range(ME):
                    p1 = ps1.tile([P, TILE_T], F32, tag="p1")
                    for k in range(KD):
                        nc.tensor.matmul(
                            p1[:, :T],
                            lhsT=wpt_sb[:, k, m * P : (m + 1) * P],
                            rhs=xnT[:, k],
                            start=(k == 0),
                            stop=(k == KD - 1),
                        )
                    nc.scalar.activation(
                        out=projT[:, m, :T],
                        in_=p1[:, :T],
                        func=mybir.ActivationFunctionType.Silu,
                        bias=bpm_sb[:, m : m + 1],
                        scale=1.0,
                    )
                # matmul 2: [D_STATE, T]; SiLU+(b_state+init) fused
                p2 = ps2.tile([D_STATE, TILE_T], F32, tag="p2")
                for k2 in range(ME):
                    nc.tensor.matmul(
                        p2[:, :T],
                        lhsT=wst_sb[:, k2, :],
                        rhs=projT[:, k2, :T],
                        start=(k2 == 0),
                        stop=(k2 == ME - 1),
                    )
                # next tile's transposes fill the PE while ACT drains
                # p2 -> cs9; LN for the tile after runs on DVE behind them
                if i + 1 < len(tiles):
                    xnT_cur = a_tr(xn_next, tiles[i + 1][1])
                if i + 2 < len(tiles):
                    xn_next = a_ln(x_tiles[i + 2], tiles[i + 2][1])
                nc.scalar.activation(
                    out=cs9[:D_STATE, :T],
                    in_=p2[:, :T],
                    func=mybir.ActivationFunctionType.Silu,
                    bias=b2_sb,
                    scale=1.0,
                )
                # matmul 3: K=9 (ones row adds b_out), token-major out;
                # residual add fused into the DVE copyback
                out_sb = outp.tile([P, G, D_MODEL], F32, tag="out")
                for g in range(G):
                    for h in range(D_MODEL // 512):
                        p3 = ps3.tile([P, 512], F32, tag="p3")
                        nc.tensor.matmul(
                            p3,
                            lhsT=cs9[:, g * P : (g + 1) * P],
                            rhs=wo9_sb[:, h * 512 : (h + 1) * 512],
                            start=True,
                            stop=True,
                        )
                        nc.vector.tensor_add(
                            out=out_sb[:, g, h * 512 : (h + 1) * 512],
                            in0=p3,
                            in1=x_sb[:, g, h * 512 : (h + 1) * 512],
                        )
                nc.sync.dma_start(
                    y[off : off + T, :].rearrange("(g p) d -> p g d", p=P), out_sb
                )

    _split_multi_waits(nc)
    return nc


_NC_CACHE = None


def _get_nc():
    global _NC_CACHE
    if _NC_CACHE is None:
        _NC_CACHE = build_kernel()
    return _NC_CACHE


def make_in_maps(inputs):
    x = np.ascontiguousarray(inputs["x"], dtype=np.float32).reshape(-1, D_MODEL)
    W_proj = np.asarray(inputs["W_proj"], dtype=np.float32)
    b_proj = np.asarray(inputs["b_proj"], dtype=np.float32)
    W_state = np.asarray(inputs["W_state"], dtype=np.float32)
    b_state = np.asarray(inputs["b_state"], dtype=np.float32)
    W_out = np.asarray(inputs["W_out"], dtype=np.float32)
    b_out = np.asarray(inputs["b_out"], dtype=np.float32)
    initial_state = np.asarray(inputs["initial_state"], dtype=np.float32)
    gamma = np.asarray(inputs["gamma"], dtype=np.float32)
    beta = np.asarray(inputs["beta"], dtype=np.float32)

    shared = {
        "wpt": np.ascontiguousarray(W_proj.T),
        "wst": np.ascontiguousarray(W_state.T),
        "wo9": np.ascontiguousarray(
            np.concatenate([W_out.T, b_out[None, :]], axis=0)
        ),
        "gpk": np.ascontiguousarray(gamma.reshape(KD, P).T),
        "bpk": np.ascontiguousarray(beta.reshape(KD, P).T),
        "bpm": np.ascontiguousarray(b_proj.reshape(ME, P).T),
        "b2": np.ascontiguousarray(
            (b_state + initial_state.reshape(-1)).reshape(D_STATE, 1)
        ),
        "ones": np.ones((1, TILE_T), dtype=np.float32),
        "ident": np.eye(P, dtype=np.float32),
    }
    in_maps = []
    for c in range(N_CORES):
        m = {"x": np.ascontiguousarray(x[c * TOK : (c + 1) * TOK])}
        m.update(shared)
        in_maps.append(m)
    return in_maps


def kernel(**inputs) -> np.ndarray:
    nc = _get_nc()
    in_maps = make_in_maps(inputs)
    res = run_bass_kernel_spmd(nc, in_maps, core_ids=list(range(N_CORES)))
    out = np.concatenate([res.results[c]["y"] for c in range(N_CORES)], axis=0)
    return out.reshape(np.asarray(inputs["x"]).shape)



# revision 6
# speedup vs baseline: 1.2123x; 1.2123x over previous
"""Trainium2 Bass kernel for the MemoryEfficientMambaBlock problem.

Data-parallel over 8 NeuronCores: x sharded over tokens, small weights
replicated. Per core, per 448-token tile (14 tiles exactly cover the
6272 tokens/core; tokens grouped 4x112 so the DMA-XBAR transpose tiles
align):
  LayerNorm (bn_stats token-major, batched sqrt on ACT) -> DVE
  tensor_scalar writes xhat in bf16 (gamma folded into W_proj, beta into
  the proj bias) -> DMA-XBAR transpose to feature-major bf16 (no PE
  transposes) -> bf16 matmul x@W_projT with SiLU+bias fused in the ACT
  copyback -> bf16 matmul @W_stateT -> SiLU+(b_state+initial_state) ->
  K=9 bf16 matmul (ones row carries b_out) producing token-major output;
  residual add on DVE. M3 of tile i runs at the top of iteration i+1 so
  the PE never waits on the cs9 SiLU.
"""

import sys

if "/opt/trn_rl_repo" not in sys.path:
    sys.path.insert(0, "/opt/trn_rl_repo")

import ml_dtypes
import numpy as np

import concourse.bass as bass
import concourse.mybir as mybir
import concourse.tile as tile
from concourse.bass_utils import run_bass_kernel_spmd

P = 128
PG = 112  # tokens per partition-group (multiple of 16 for the XBAR)
G = 4  # groups per tile
TILE_T = PG * G  # 448
D_MODEL = 1024
D_INNER = 2048
D_STATE = 8
EPS = 1e-5
N_CORES = 8
TOK_TOTAL = 2 * 128 * 196  # 50176
TOK = TOK_TOTAL // N_CORES  # 6272
NT = TOK // TILE_T  # 14 tiles exactly

KD = D_MODEL // P  # 8 contraction chunks for matmul 1
ME = D_INNER // P  # 16 output tiles for matmul 1 / contraction chunks for 2

F32 = mybir.dt.float32
BF16 = mybir.dt.bfloat16


def _split_multi_waits(nc):
    """This container's walrus accepts at most ONE semaphore wait per
    instruction. Hoist all but the last wait of each instruction onto
    fresh single-wait NoOps inserted immediately before it on the same
    engine (the sequencer processes instructions in order, so semantics
    are unchanged)."""
    n_split = 0
    for f in nc.m.functions:
        for blk in f.blocks:
            out = []
            changed = False
            for inst in blk.instructions:
                si = inst.sync_info
                waits = list(si.on_wait) if si is not None else []
                if len(waits) > 1:
                    changed = True
                    for j, w in enumerate(waits[:-1]):
                        nop = mybir.InstNoOp(
                            name=f"{inst.name}-wsplit{j}", ins=[], outs=[]
                        )
                        nop.engine = inst.engine
                        nop.sync_info = mybir.SyncInfo(on_wait=[w], on_update=[])
                        out.append(nop)
                        n_split += 1
                    inst.sync_info = mybir.SyncInfo(
                        on_wait=[waits[-1]], on_update=list(si.on_update)
                    )
                out.append(inst)
            if changed:
                blk.instructions = out
    return n_split


def build_kernel():
    nc = bass.Bass()
    x = nc.dram_tensor("x", [TOK, D_MODEL], F32, kind="ExternalInput")
    # [m, p, c, j] = (W_proj*gamma)[m*128+j, c*128+p]
    wpt = nc.dram_tensor("wpt", [ME, P, KD, P], BF16, kind="ExternalInput")
    wst = nc.dram_tensor("wst", [P, ME, D_STATE], BF16, kind="ExternalInput")
    wo9 = nc.dram_tensor("wo9", [D_STATE + 1, D_MODEL], BF16, kind="ExternalInput")
    bpm = nc.dram_tensor("bpm", [P, ME], F32, kind="ExternalInput")
    b2 = nc.dram_tensor("b2", [D_STATE, 1], F32, kind="ExternalInput")
    y = nc.dram_tensor("y", [TOK, D_MODEL], F32, kind="ExternalOutput")

    tiles = [(i * TILE_T, TILE_T) for i in range(NT)]

    with tile.TileContext(nc) as tc:
        with (
            tc.tile_pool(name="singles", bufs=1) as singles,
            tc.tile_pool(name="xpool", bufs=3) as xpool,
            tc.tile_pool(name="xnpool", bufs=2) as xnpool,
            tc.tile_pool(name="xtpool", bufs=2) as xtpool,
            tc.tile_pool(name="projp", bufs=2) as projp,
            tc.tile_pool(name="outp", bufs=2) as outp,
            tc.tile_pool(name="statp", bufs=6) as statp,
            tc.tile_pool(name="ps1", bufs=3, space="PSUM") as ps1,
            tc.tile_pool(name="ps2", bufs=1, space="PSUM") as ps2,
            tc.tile_pool(name="ps3", bufs=4, space="PSUM") as ps3,
        ):
            # x tiles 0 and 1 first so LayerNorm can start ASAP; weight
            # slices stream in behind them on both DMA queues.
            def a_dma(off, _T):
                x_sb = xpool.tile([PG, G, D_MODEL], F32, tag="x")
                nc.sync.dma_start(
                    x_sb, x[off : off + TILE_T, :].rearrange("(g p) d -> p g d", p=PG)
                )
                return x_sb

            x_tiles = [a_dma(*tiles[0])]

            # weights: per-m slices, alternating queues, so matmul-1 can
            # begin after the first slice lands
            wpt_sb = singles.tile([P, ME, KD, P], BF16)
            for m in range(ME):
                eng = nc.sync if m % 2 == 0 else nc.scalar
                eng.dma_start(wpt_sb[:, m], wpt[m])
            x_tiles.append(a_dma(*tiles[1]))
            wst_sb = singles.tile([P, ME, D_STATE], BF16)
            nc.scalar.dma_start(wst_sb, wst[:, :])
            wo9_sb = singles.tile([D_STATE + 1, D_MODEL], BF16)
            nc.scalar.dma_start(wo9_sb, wo9[:, :])
            bpm_sb = singles.tile([P, ME], F32)
            nc.scalar.dma_start(bpm_sb, bpm[:, :])
            b2_sb = singles.tile([D_STATE, 1], F32)
            nc.scalar.dma_start(b2_sb, b2[:, :])
            eps_sb = singles.tile([PG, 1], F32)
            nc.vector.memset(eps_sb, EPS)
            # cs9 double buffer; row 8 is the constant 1.0 that multiplies
            # the b_out row of wo9
            cs9_bufs = [
                singles.tile(
                    [D_STATE + 1, TILE_T], BF16, tag=f"cs9{j}", name=f"cs9{j}"
                )
                for j in range(2)
            ]
            for j in range(2):
                # whole-tile memset (partition-8-based APs are illegal);
                # rows 0..7 get overwritten by the SiLU each tile, row 8
                # stays 1.0 and multiplies the b_out row of wo9
                nc.gpsimd.memset(cs9_bufs[j], 1.0)

            def a_ln(x_sb):
                """LayerNorm stats + normalize -> bf16 token-major"""
                mvt = statp.tile([PG, G, 2], F32, tag="mv")
                for g in range(G):
                    stats = statp.tile([PG, 2, 6], F32, tag="bnst")
                    nc.vector.bn_stats(stats[:, 0, :], x_sb[:, g, 0:512])
                    nc.vector.bn_stats(stats[:, 1, :], x_sb[:, g, 512:1024])
                    nc.vector.bn_aggr(mvt[:, g], stats)
                rstd = statp.tile([PG, G], F32, tag="rstd")
                nc.scalar.activation(
                    rstd,
                    mvt[:, :, 1],
                    mybir.ActivationFunctionType.Sqrt,
                    bias=eps_sb,
                )
                nc.vector.reciprocal(rstd, rstd)
                xn = xnpool.tile([PG, G, D_MODEL], BF16, tag="xn")
                for g in range(G):
                    nc.vector.tensor_scalar(
                        out=xn[:, g],
                        in0=x_sb[:, g],
                        scalar1=mvt[:, g, 0:1],
                        scalar2=rstd[:, g : g + 1],
                        op0=mybir.AluOpType.subtract,
                        op1=mybir.AluOpType.mult,
                    )
                return xn

            def a_tr(xn):
                """DMA-XBAR transpose to feature-major: [p, g, c, t] with
                feature d = c*128 + p"""
                xnT = xtpool.tile([P, G, KD, PG], BF16, tag="xnT")
                for g in range(G):
                    nc.sync.dma_start_transpose(xnT[:, g], xn[:, g, :])
                return xnT

            xn_cur = a_ln(x_tiles[0])
            xnT_cur = a_tr(xn_cur)

            # carried state for the software-pipelined matmul-3
            prev = None  # (cs9, x_sb, off)

            for i, (off, T) in enumerate(tiles):
                x_sb = x_tiles[i]
                xnT = xnT_cur
                cs9 = cs9_bufs[i % 2]

                # matmul 3 of the PREVIOUS tile: cs9 is long ready, so the
                # PE flows M3(i-1) -> M1(i) without waiting on ACT
                if prev is not None:
                    pcs9, px_sb, poff = prev
                    pout = outp.tile([PG, G, D_MODEL], F32, tag="out")
                    for g in range(G):
                        for h in range(2):
                            p3 = ps3.tile([PG, 512], F32, tag="p3")
                            nc.tensor.matmul(
                                p3,
                                lhsT=pcs9[:, g * PG : (g + 1) * PG],
                                rhs=wo9_sb[:, h * 512 : (h + 1) * 512],
                                start=True,
                                stop=True,
                            )
                            nc.vector.tensor_add(
                                out=pout[:, g, h * 512 : (h + 1) * 512],
                                in0=p3,
                                in1=px_sb[:, g, h * 512 : (h + 1) * 512],
                            )
                    nc.scalar.dma_start(
                        y[poff : poff + TILE_T, :].rearrange(
                            "(g p) d -> p g d", p=PG
                        ),
                        pout,
                    )

                # x two tiles ahead (after M3(i-1): its residual reads of
                # x(i-1) must be emitted before this DMA reuses the buffer)
                if i + 2 < NT:
                    x_tiles.append(a_dma(*tiles[i + 2]))

                # matmul 1: [D_INNER, T] feature-major; SiLU+bias fused
                projT = projp.tile([P, ME, TILE_T], BF16, tag="projT")
                for m in range(ME):
                    p1 = ps1.tile([P, TILE_T], F32, tag="p1")
                    for c in range(KD):
                        nc.tensor.matmul(
                            p1,
                            lhsT=wpt_sb[:, m, c, :],
                            rhs=xnT[:, :, c, :],
                            start=(c == 0),
                            stop=(c == KD - 1),
                        )
                    nc.scalar.activation(
                        out=projT[:, m],
                        in_=p1,
                        func=mybir.ActivationFunctionType.Silu,
                        bias=bpm_sb[:, m : m + 1],
                        scale=1.0,
                    )

                # next tile's LayerNorm + transpose run on DVE/DMA while
                # the PE streams matmul 1/2
                if i + 1 < NT:
                    xn_next = a_ln(x_tiles[i + 1])
                    xnT_cur = a_tr(xn_next)

                # matmul 2: [D_STATE, T]; SiLU+(b_state+init) fused
                p2 = ps2.tile([D_STATE, TILE_T], F32, tag="p2")
                for k2 in range(ME):
                    nc.tensor.matmul(
                        p2,
                        lhsT=wst_sb[:, k2, :],
                        rhs=projT[:, k2, :],
                        start=(k2 == 0),
                        stop=(k2 == ME - 1),
                    )
                nc.scalar.activation(
                    out=cs9[:D_STATE, :],
                    in_=p2,
                    func=mybir.ActivationFunctionType.Silu,
                    bias=b2_sb,
                    scale=1.0,
                )
                prev = (cs9, x_sb, off)

            # epilogue: matmul 3 of the last tile
            pcs9, px_sb, poff = prev
            pout = outp.tile([PG, G, D_MODEL], F32, tag="out")
            for g in range(G):
                for h in range(2):
                    p3 = ps3.tile([PG, 512], F32, tag="p3")
                    nc.tensor.matmul(
                        p3,
                        lhsT=pcs9[:, g * PG : (g + 1) * PG],
                        rhs=wo9_sb[:, h * 512 : (h + 1) * 512],
                        start=True,
                        stop=True,
                    )
                    nc.vector.tensor_add(
                        out=pout[:, g, h * 512 : (h + 1) * 512],
                        in0=p3,
                        in1=px_sb[:, g, h * 512 : (h + 1) * 512],
                    )
            nc.scalar.dma_start(
                y[poff : poff + TILE_T, :].rearrange("(g p) d -> p g d", p=PG),
                pout,
            )

    _split_multi_waits(nc)
    return nc


_NC_CACHE = None


def _get_nc():
    global _NC_CACHE
    if _NC_CACHE is None:
        _NC_CACHE = build_kernel()
    return _NC_CACHE


def make_in_maps(inputs):
    x = np.ascontiguousarray(inputs["x"], dtype=np.float32).reshape(-1, D_MODEL)
    W_proj = np.asarray(inputs["W_proj"], dtype=np.float32)
    b_proj = np.asarray(inputs["b_proj"], dtype=np.float32)
    W_state = np.asarray(inputs["W_state"], dtype=np.float32)
    b_state = np.asarray(inputs["b_state"], dtype=np.float32)
    W_out = np.asarray(inputs["W_out"], dtype=np.float32)
    b_out = np.asarray(inputs["b_out"], dtype=np.float32)
    initial_state = np.asarray(inputs["initial_state"], dtype=np.float32)
    gamma = np.asarray(inputs["gamma"], dtype=np.float32)
    beta = np.asarray(inputs["beta"], dtype=np.float32)

    # gamma folds into W_proj, beta into the proj bias
    Wg = W_proj * gamma[None, :]
    bvec = b_proj + W_proj @ beta
    # [m, p, c, j] = Wg[m*128+j, c*128+p]
    wpt_host = np.ascontiguousarray(
        Wg.reshape(ME, P, KD, P).transpose(0, 3, 2, 1)
    ).astype(ml_dtypes.bfloat16)
    wst_host = np.ascontiguousarray(
        W_state.T.reshape(ME, P, D_STATE).transpose(1, 0, 2)
    ).astype(ml_dtypes.bfloat16)

    shared = {
        "wpt": wpt_host,
        "wst": wst_host,
        "wo9": np.ascontiguousarray(
            np.concatenate([W_out.T, b_out[None, :]], axis=0)
        ).astype(ml_dtypes.bfloat16),
        "bpm": np.ascontiguousarray(bvec.reshape(ME, P).T),
        "b2": np.ascontiguousarray(
            (b_state + initial_state.reshape(-1)).reshape(D_STATE, 1)
        ),
    }
    in_maps = []
    for c in range(N_CORES):
        m = {"x": np.ascontiguousarray(x[c * TOK : (c + 1) * TOK])}
        m.update(shared)
        in_maps.append(m)
    return in_maps


def kernel(**inputs) -> np.ndarray:
    nc = _get_nc()
    in_maps = make_in_maps(inputs)
    res = run_bass_kernel_spmd(nc, in_maps, core_ids=list(range(N_CORES)))
    out = np.concatenate([res.results[c]["y"] for c in range(N_CORES)], axis=0)
    return out.reshape(np.asarray(inputs["x"]).shape)


# revision 7
# speedup vs baseline: 1.2617x; 1.0407x over previous
"""Trainium2 Bass kernel for the MemoryEfficientMambaBlock problem.

Data-parallel over 8 NeuronCores: x sharded over tokens, small weights
replicated. Per core, per 448-token tile (14 tiles exactly cover the
6272 tokens/core; tokens grouped 4x112 so the DMA-XBAR transpose tiles
align):
  LayerNorm (bn_stats token-major, batched sqrt on ACT) -> DVE
  tensor_scalar writes xhat in bf16 (gamma folded into W_proj, beta into
  the proj bias) -> DMA-XBAR transpose to feature-major bf16 (no PE
  transposes) -> bf16 matmul x@W_projT with SiLU+bias fused in the ACT
  copyback -> bf16 matmul @W_stateT -> SiLU+(b_state+initial_state) ->
  K=9 bf16 matmul (ones row carries b_out), residual added in place into
  the x tile on DVE.

Pipelining: the LN+transpose chain for tile i+2 runs during tile i
(2-deep, so the PE never waits on it), x tiles DMA as halves on both
HWDGE queues, y writes go out on the gpsimd queue, and tile i-1's eight
matmul-3's are interleaved between tile i's matmul-1 m-steps so their
PSUM drains (DVE residual adds) never pace the PE.
"""

import sys

if "/opt/trn_rl_repo" not in sys.path:
    sys.path.insert(0, "/opt/trn_rl_repo")

import ml_dtypes
import numpy as np

import concourse.bass as bass
import concourse.mybir as mybir
import concourse.tile as tile
from concourse.bass_utils import run_bass_kernel_spmd

P = 128
PG = 112  # tokens per partition-group (multiple of 16 for the XBAR)
G = 4  # groups per tile
TILE_T = PG * G  # 448
D_MODEL = 1024
D_INNER = 2048
D_STATE = 8
EPS = 1e-5
N_CORES = 8
TOK_TOTAL = 2 * 128 * 196  # 50176
TOK = TOK_TOTAL // N_CORES  # 6272
NT = TOK // TILE_T  # 14 tiles exactly

KD = D_MODEL // P  # 8 contraction chunks for matmul 1
ME = D_INNER // P  # 16 output tiles for matmul 1 / contraction chunks for 2

F32 = mybir.dt.float32
BF16 = mybir.dt.bfloat16


def _split_multi_waits(nc):
    """This container's walrus accepts at most ONE semaphore wait per
    instruction. Hoist all but the last wait of each instruction onto
    fresh single-wait NoOps inserted immediately before it on the same
    engine (the sequencer processes instructions in order, so semantics
    are unchanged)."""
    n_split = 0
    for f in nc.m.functions:
        for blk in f.blocks:
            out = []
            changed = False
            for inst in blk.instructions:
                si = inst.sync_info
                waits = list(si.on_wait) if si is not None else []
                if len(waits) > 1:
                    changed = True
                    for j, w in enumerate(waits[:-1]):
                        nop = mybir.InstNoOp(
                            name=f"{inst.name}-wsplit{j}", ins=[], outs=[]
                        )
                        nop.engine = inst.engine
                        nop.sync_info = mybir.SyncInfo(on_wait=[w], on_update=[])
                        out.append(nop)
                        n_split += 1
                    inst.sync_info = mybir.SyncInfo(
                        on_wait=[waits[-1]], on_update=list(si.on_update)
                    )
                out.append(inst)
            if changed:
                blk.instructions = out
    return n_split


def build_kernel():
    nc = bass.Bass()
    x = nc.dram_tensor("x", [TOK, D_MODEL], F32, kind="ExternalInput")
    # [m, p, c, j] = (W_proj*gamma)[m*128+j, c*128+p]
    wpt = nc.dram_tensor("wpt", [ME, P, KD, P], BF16, kind="ExternalInput")
    wst = nc.dram_tensor("wst", [P, ME, D_STATE], BF16, kind="ExternalInput")
    wo9 = nc.dram_tensor("wo9", [D_STATE + 1, D_MODEL], BF16, kind="ExternalInput")
    bpm = nc.dram_tensor("bpm", [P, ME], F32, kind="ExternalInput")
    b2 = nc.dram_tensor("b2", [D_STATE, 1], F32, kind="ExternalInput")
    y = nc.dram_tensor("y", [TOK, D_MODEL], F32, kind="ExternalOutput")

    with tile.TileContext(nc) as tc:
        with (
            tc.tile_pool(name="singles", bufs=1) as singles,
            tc.tile_pool(name="xpool", bufs=4) as xpool,
            tc.tile_pool(name="xnpool", bufs=2) as xnpool,
            tc.tile_pool(name="xtpool", bufs=3) as xtpool,
            tc.tile_pool(name="projp", bufs=2) as projp,
            tc.tile_pool(name="statp", bufs=6) as statp,
            tc.tile_pool(name="ps1", bufs=3, space="PSUM") as ps1,
            tc.tile_pool(name="ps2", bufs=1, space="PSUM") as ps2,
            tc.tile_pool(name="ps3", bufs=2, space="PSUM") as ps3,
        ):

            def a_dma(i):
                """x tile as two halves, one per HWDGE queue"""
                off = i * TILE_T
                x_sb = xpool.tile([PG, G, D_MODEL], F32, tag="x")
                half = TILE_T // 2
                nc.sync.dma_start(
                    x_sb[:, 0:2],
                    x[off : off + half, :].rearrange("(g p) d -> p g d", p=PG),
                )
                nc.scalar.dma_start(
                    x_sb[:, 2:4],
                    x[off + half : off + TILE_T, :].rearrange(
                        "(g p) d -> p g d", p=PG
                    ),
                )
                return x_sb

            x_tiles = [a_dma(0), a_dma(1)]

            wst_sb = singles.tile([P, ME, D_STATE], BF16)
            nc.scalar.dma_start(wst_sb, wst[:, :])
            wo9_sb = singles.tile([D_STATE + 1, D_MODEL], BF16)
            nc.scalar.dma_start(wo9_sb, wo9[:, :])
            bpm_sb = singles.tile([P, ME], F32)
            nc.scalar.dma_start(bpm_sb, bpm[:, :])
            b2_sb = singles.tile([D_STATE, 1], F32)
            nc.scalar.dma_start(b2_sb, b2[:, :])
            eps_sb = singles.tile([PG, 1], F32)
            nc.vector.memset(eps_sb, EPS)
            cs9_bufs = [
                singles.tile(
                    [D_STATE + 1, TILE_T], BF16, tag=f"cs9{j}", name=f"cs9{j}"
                )
                for j in range(2)
            ]
            for j in range(2):
                # whole-tile memset (partition-8-based APs are illegal);
                # rows 0..7 get overwritten by the SiLU each tile, row 8
                # stays 1.0 and multiplies the b_out row of wo9
                nc.gpsimd.memset(cs9_bufs[j], 1.0)

            def a_stats(x_sb):
                """bn stats for all 4 groups -> [PG, G, 2] mean/var"""
                mvt = statp.tile([PG, G, 2], F32, tag="mv")
                for g in range(G):
                    stats = statp.tile([PG, 2, 6], F32, tag="bnst")
                    nc.vector.bn_stats(stats[:, 0, :], x_sb[:, g, 0:512])
                    nc.vector.bn_stats(stats[:, 1, :], x_sb[:, g, 512:1024])
                    nc.vector.bn_aggr(mvt[:, g], stats)
                return mvt

            def a_norm(x_sb, mvt):
                """rstd (batched sqrt) + normalize -> bf16 token-major"""
                rstd = statp.tile([PG, G], F32, tag="rstd")
                nc.scalar.activation(
                    rstd,
                    mvt[:, :, 1],
                    mybir.ActivationFunctionType.Sqrt,
                    bias=eps_sb,
                )
                nc.vector.reciprocal(rstd, rstd)
                xn = xnpool.tile([PG, G, D_MODEL], BF16, tag="xn")
                for g in range(G):
                    nc.vector.tensor_scalar(
                        out=xn[:, g],
                        in0=x_sb[:, g],
                        scalar1=mvt[:, g, 0:1],
                        scalar2=rstd[:, g : g + 1],
                        op0=mybir.AluOpType.subtract,
                        op1=mybir.AluOpType.mult,
                    )
                return xn

            def a_tr(xn):
                """DMA-XBAR transpose to feature-major: [p, g, c, t] with
                feature d = c*128 + p"""
                xnT = xtpool.tile([P, G, KD, PG], BF16, tag="xnT")
                for g in range(G):
                    nc.sync.dma_start_transpose(xnT[:, g], xn[:, g, :])
                return xnT

            def m3_step(pcs9, px_sb, g):
                """one group of the previous tile's matmul 3 + in-place
                residual; the y DMA for the group goes out right away"""
                for h in range(2):
                    p3 = ps3.tile([PG, 512], F32, tag="p3")
                    nc.tensor.matmul(
                        p3,
                        lhsT=pcs9[:, g * PG : (g + 1) * PG],
                        rhs=wo9_sb[:, h * 512 : (h + 1) * 512],
                        start=True,
                        stop=True,
                    )
                    nc.vector.tensor_add(
                        out=px_sb[:, g, h * 512 : (h + 1) * 512],
                        in0=p3,
                        in1=px_sb[:, g, h * 512 : (h + 1) * 512],
                    )

            def y_dma(px_sb, poff):
                nc.gpsimd.dma_start(
                    y[poff : poff + TILE_T, :].rearrange("(g p) d -> p g d", p=PG),
                    px_sb,
                )

            # prologue: weights after the first x tiles; LN chains for
            # tiles 0 and 1
            wpt_sb = singles.tile([P, ME, KD, P], BF16)
            mvt0 = a_stats(x_tiles[0])
            xnT_q = [None, None]  # xnT for tiles i+1, i+2 relative to loop
            xn0 = a_norm(x_tiles[0], mvt0)
            xnT_q[0] = a_tr(xn0)
            for m in range(ME):
                eng = nc.sync if m % 2 == 0 else nc.scalar
                eng.dma_start(wpt_sb[:, m], wpt[m])
            mvt1 = a_stats(x_tiles[1])
            xn1 = a_norm(x_tiles[1], mvt1)
            xnT_q[1] = a_tr(xn1)

            prev = None  # (cs9, x_sb, off) of tile i-1

            for i in range(NT):
                off = i * TILE_T
                x_sb = x_tiles[i]
                xnT = xnT_q[0]
                xnT_q[0] = xnT_q[1]
                cs9 = cs9_bufs[i % 2]
                if i + 2 < NT:
                    x_tiles.append(a_dma(i + 2))

                # matmul 1 m-steps with the previous tile's matmul-3
                # groups interleaved between them
                projT = projp.tile([P, ME, TILE_T], BF16, tag="projT")
                for m in range(ME):
                    p1 = ps1.tile([P, TILE_T], F32, tag="p1")
                    for c in range(KD):
                        nc.tensor.matmul(
                            p1,
                            lhsT=wpt_sb[:, m, c, :],
                            rhs=xnT[:, :, c, :],
                            start=(c == 0),
                            stop=(c == KD - 1),
                        )
                    nc.scalar.activation(
                        out=projT[:, m],
                        in_=p1,
                        func=mybir.ActivationFunctionType.Silu,
                        bias=bpm_sb[:, m : m + 1],
                        scale=1.0,
                    )
                    if prev is not None and m % 2 == 1 and m // 2 < G:
                        m3_step(prev[0], prev[1], m // 2)
                if prev is not None:
                    y_dma(prev[1], prev[2])

                # LN chain for tile i+2 (2-deep pipeline)
                if i + 2 < NT:
                    mvt_n = a_stats(x_tiles[i + 2])

                # matmul 2 + cs9 SiLU
                p2 = ps2.tile([D_STATE, TILE_T], F32, tag="p2")
                for k2 in range(ME):
                    nc.tensor.matmul(
                        p2,
                        lhsT=wst_sb[:, k2, :],
                        rhs=projT[:, k2, :],
                        start=(k2 == 0),
                        stop=(k2 == ME - 1),
                    )
                nc.scalar.activation(
                    out=cs9[:D_STATE, :],
                    in_=p2,
                    func=mybir.ActivationFunctionType.Silu,
                    bias=b2_sb,
                    scale=1.0,
                )
                if i + 2 < NT:
                    xn_n = a_norm(x_tiles[i + 2], mvt_n)
                    xnT_q[1] = a_tr(xn_n)
                prev = (cs9, x_sb, off)

            # epilogue: matmul 3 of the last tile
            pcs9, px_sb, poff = prev
            for g in range(G):
                m3_step(pcs9, px_sb, g)
            y_dma(px_sb, poff)

    _split_multi_waits(nc)
    return nc


_NC_CACHE = None


def _get_nc():
    global _NC_CACHE
    if _NC_CACHE is None:
        _NC_CACHE = build_kernel()
    return _NC_CACHE


def make_in_maps(inputs):
    x = np.ascontiguousarray(inputs["x"], dtype=np.float32).reshape(-1, D_MODEL)
    W_proj = np.asarray(inputs["W_proj"], dtype=np.float32)
    b_proj = np.asarray(inputs["b_proj"], dtype=np.float32)
    W_state = np.asarray(inputs["W_state"], dtype=np.float32)
    b_state = np.asarray(inputs["b_state"], dtype=np.float32)
    W_out = np.asarray(inputs["W_out"], dtype=np.float32)
    b_out = np.asarray(inputs["b_out"], dtype=np.float32)
    initial_state = np.asarray(inputs["initial_state"], dtype=np.float32)
    gamma = np.asarray(inputs["gamma"], dtype=np.float32)
    beta = np.asarray(inputs["beta"], dtype=np.float32)

    # gamma folds into W_proj, beta into the proj bias
    Wg = W_proj * gamma[None, :]
    bvec = b_proj + W_proj @ beta
    # [m, p, c, j] = Wg[m*128+j, c*128+p]
    wpt_host = np.ascontiguousarray(
        Wg.reshape(ME, P, KD, P).transpose(0, 3, 2, 1)
    ).astype(ml_dtypes.bfloat16)
    wst_host = np.ascontiguousarray(
        W_state.T.reshape(ME, P, D_STATE).transpose(1, 0, 2)
    ).astype(ml_dtypes.bfloat16)

    shared = {
        "wpt": wpt_host,
        "wst": wst_host,
        "wo9": np.ascontiguousarray(
            np.concatenate([W_out.T, b_out[None, :]], axis=0)
        ).astype(ml_dtypes.bfloat16),
        "bpm": np.ascontiguousarray(bvec.reshape(ME, P).T),
        "b2": np.ascontiguousarray(
            (b_state + initial_state.reshape(-1)).reshape(D_STATE, 1)
        ),
    }
    in_maps = []
    for c in range(N_CORES):
        m = {"x": np.ascontiguousarray(x[c * TOK : (c + 1) * TOK])}
        m.update(shared)
        in_maps.append(m)
    return in_maps


def kernel(**inputs) -> np.ndarray:
    nc = _get_nc()
    in_maps = make_in_maps(inputs)
    res = run_bass_kernel_spmd(nc, in_maps, core_ids=list(range(N_CORES)))
    out = np.concatenate([res.results[c]["y"] for c in range(N_CORES)], axis=0)
    return out.reshape(np.asarray(inputs["x"]).shape)
